# revision 4
# baseline (speedup 1.0000x reference)
"""MoE top-1 routing (fairseq Top1Gate style) on 8 trn2 NeuronCores.

Strategy:
  - Host (numpy, float64): gate logits, softmax, argmax, capacity cumsum,
    l_aux.  This is the shard-construction step: it produces, for each
    expert, the list of kept tokens and their capacity slots.
  - Shard: expert-parallel x token-parallel.  Core 2e+j handles expert e's
    capacity slots [j*1024, (j+1)*1024).  Each core receives its tokens
    already gathered AND transposed ([M, 1024] bf16) plus its expert's
    fc1/fc2 weights (bf16) and fc1 bias.
  - Device (Bass/Tile, bf16 matmuls, fp32 accum):
      hT = gelu_tanh(W1.T-contracted xt + b1)   # [H, tok] layout
      out = hT.T-contracted W2                  # [tok, O] fp32
  - Host combine: out_full[token] = gate * (core_out[slot] + fc2_b[e]).

Self-contained: shapes hardcoded for B=8,T=1024,M=1024,H=4096,O=1024,E=4.
"""

import os
import numpy as np
import ml_dtypes

B, T, M, H, O, E = 8, 1024, 1024, 4096, 1024, 4
S = B * T            # 8192 tokens
C = 2048             # capacity = ceil(S/E) * 1.0
NCORES = 8
TOK = C // 2         # tokens per core
P = 128

KM = M // P          # 8  k-chunks for GEMM1
NH = H // P          # 32 h-tiles
HC = 4               # w1/w2 stream chunks
NH_C = NH // HC      # 8  h-tiles per chunk

LAST_RESULTS = None  # test.py introspection (BassKernelResults)
_NC_CACHE = None

BF16 = ml_dtypes.bfloat16


# --------------------------------------------------------------------------
# Host routing (the gate + shard construction)
# --------------------------------------------------------------------------

def _route(features, gate_w):
    """Returns (idx, keep, gate_val, l_aux, per-expert token lists)."""
    lg = features.astype(np.float64) @ gate_w.astype(np.float64)      # [S, E]
    idx = np.argmax(lg, axis=1)
    z = lg - lg.max(axis=1, keepdims=True)
    ez = np.exp(z)
    gates = ez / ez.sum(axis=1, keepdims=True)                        # [S, E]

    mask1 = np.zeros((S, E), np.float64)
    mask1[np.arange(S), idx] = 1.0
    me = gates.mean(axis=0)
    ce = mask1.mean(axis=0)
    l_aux = np.float32((me * ce).sum() * E)

    # position of each token within its expert's queue (token order)
    locations = np.cumsum(mask1, axis=0) - 1.0
    loc = locations[np.arange(S), idx].astype(np.int64)
    keep = loc < C
    gate_val = (gates[np.arange(S), idx] * keep).astype(np.float32)

    toks_per_e = [np.nonzero((idx == e) & keep)[0] for e in range(E)]
    return idx, keep, gate_val, l_aux, toks_per_e


# --------------------------------------------------------------------------
# Device kernel (Bass/Tile): per-core expert MLP chunk
# --------------------------------------------------------------------------

def _declare_params(nc, mybir):
    F32 = mybir.dt.float32
    DBF16 = mybir.dt.bfloat16
    xt = nc.declare_dram_parameter("xt", [M, TOK], DBF16, isOutput=False)
    w1 = nc.declare_dram_parameter("w1", [M, H], DBF16, isOutput=False)
    b1 = nc.declare_dram_parameter("b1", [P, NH], F32, isOutput=False)
    w2 = nc.declare_dram_parameter("w2", [H, O], DBF16, isOutput=False)
    out = nc.declare_dram_parameter("out", [TOK, O], F32, isOutput=True)
    return xt, w1, b1, w2, out


def _make_pools(tc, ctx):
    pools = {}
    pools["xt"] = ctx.enter_context(tc.tile_pool(name="xt_pool", bufs=1))
    pools["b1"] = ctx.enter_context(tc.tile_pool(name="b1_pool", bufs=1))
    pools["hT"] = ctx.enter_context(tc.tile_pool(name="hT_pool", bufs=1))
    pools["w1"] = ctx.enter_context(tc.tile_pool(name="w1_pool", bufs=2))
    pools["w2"] = ctx.enter_context(tc.tile_pool(name="w2_pool", bufs=1))
    pools["out"] = ctx.enter_context(tc.tile_pool(name="out_pool", bufs=4))
    pools["psA"] = ctx.enter_context(tc.tile_pool(name="psA", bufs=4, space="PSUM"))
    pools["psB"] = ctx.enter_context(tc.tile_pool(name="psB", bufs=3, space="PSUM"))
    return pools


def _emit_body(nc, pools, mybir, xt, w1, b1, w2, out):
    F32 = mybir.dt.float32
    DBF16 = mybir.dt.bfloat16
    GELU = mybir.ActivationFunctionType.Gelu_apprx_tanh

    xt_r = xt[:].rearrange("(k p) t -> p k t", p=P)     # [128, 8, 1024]
    w1_r = w1[:].rearrange("(k p) h -> p k h", p=P)     # [128, 8, 4096]
    w2_r = w2[:].rearrange("(k p) o -> p k o", p=P)     # [128, 32, 1024]

    # startup-critical DMAs first: xt split per k-chunk so the first
    # accumulation group's early matmuls can start after ~512KB, not 4MB
    xt_sb = pools["xt"].tile([P, KM, TOK], DBF16, tag="xt")
    for k in range(KM):
        nc.sync.dma_start(out=xt_sb[:, k, :], in_=xt_r[:, k, :])
    b1_sb = pools["b1"].tile([P, NH], F32, tag="b1")
    nc.sync.dma_start(out=b1_sb[:], in_=b1[:])

    hT = pools["hT"].tile([P, NH, TOK], DBF16, tag="hT")   # [H-part, h-tile, tok]
    w2_sb = pools["w2"].tile([P, NH, O], DBF16, tag="w2")  # resident for phase B

    # ---- phase A: hT = gelu(W1.T @ X.T + b1), H-major layout ----
    for hc in range(HC):
        w1c = pools["w1"].tile([P, KM, NH_C * P], DBF16, tag="w1c")
        if hc == 0:
            # per-k split: first matmul needs only xt[k0]+w1c[k0] resident
            for k in range(KM):
                nc.sync.dma_start(out=w1c[:, k, :], in_=w1_r[:, k, :NH_C * P])
        else:
            nc.sync.dma_start(
                out=w1c[:],
                in_=w1_r[:, :, hc * NH_C * P:(hc + 1) * NH_C * P],
            )
        # stream the matching w2 chunk behind the w1 chunk (used in phase B)
        nc.sync.dma_start(
            out=w2_sb[:, hc * NH_C:(hc + 1) * NH_C, :],
            in_=w2_r[:, hc * NH_C:(hc + 1) * NH_C, :],
        )
        for h in range(NH_C):
            h_abs = hc * NH_C + h
            for t in range(TOK // 512):
                ps = pools["psA"].tile([P, 512], F32, tag="ps")
                for k in range(KM):
                    nc.tensor.matmul(
                        ps[:],
                        lhsT=w1c[:, k, h * P:(h + 1) * P],
                        rhs=xt_sb[:, k, t * 512:(t + 1) * 512],
                        start=(k == 0),
                        stop=(k == KM - 1),
                    )
                nc.scalar.activation(
                    hT[:, h_abs, t * 512:(t + 1) * 512],
                    ps[:],
                    GELU,
                    bias=b1_sb[:, h_abs:h_abs + 1],
                )

    # ---- phase B: out = hT.T @ W2, one PSUM-bank chain per output tile ----
    # (chains complete sequentially, so each bank's drain overlaps the next
    #  bank's accumulation)
    for tt in range(TOK // P):
        for oc in range(O // 512):
            ps = pools["psB"].tile([P, 512], F32, tag="psb")
            for hk in range(NH):
                nc.tensor.matmul(
                    ps[:],
                    lhsT=hT[:, hk, tt * P:(tt + 1) * P],
                    rhs=w2_sb[:, hk, oc * 512:(oc + 1) * 512],
                    start=(hk == 0),
                    stop=(hk == NH - 1),
                )
            ot = pools["out"].tile([P, 512], F32, tag="ot")
            nc.vector.tensor_copy(ot[:], ps[:])
            nc.sync.dma_start(
                out=out[tt * P:(tt + 1) * P, oc * 512:(oc + 1) * 512],
                in_=ot[:],
            )


def _build_nc(reps=1):
    import contextlib
    import concourse.bacc as bacc
    import concourse.mybir as mybir
    from concourse import tile

    nc = bacc.Bacc(None, target_bir_lowering=False)
    xt, w1, b1, w2, out = _declare_params(nc, mybir)
    with tile.TileContext(nc) as tc, contextlib.ExitStack() as ctx:
        pools = _make_pools(tc, ctx)
        for _ in range(reps):
            _emit_body(nc, pools, mybir, xt, w1, b1, w2, out)
    nc.compile()
    return nc


def _build_nc_loop(n_iters):
    """Body wrapped in a device-side For_i loop — for benchmarking only."""
    import contextlib
    import concourse.bacc as bacc
    import concourse.mybir as mybir
    from concourse import tile

    nc = bacc.Bacc(None, target_bir_lowering=False)
    xt, w1, b1, w2, out = _declare_params(nc, mybir)
    with tile.TileContext(nc) as tc, contextlib.ExitStack() as ctx:
        pools = _make_pools(tc, ctx)
        with tc.For_i(0, n_iters, 1, hint_engines=(mybir.EngineType.PE,)):
            _emit_body(nc, pools, mybir, xt, w1, b1, w2, out)
    nc.compile()
    return nc


def _run_device(in_maps):
    global LAST_RESULTS, _NC_CACHE
    from concourse.bass_utils import run_bass_kernel_spmd

    if _NC_CACHE is None:
        _NC_CACHE = _build_nc()
    res = run_bass_kernel_spmd(_NC_CACHE, in_maps, core_ids=list(range(NCORES)))
    LAST_RESULTS = res
    return [r["out"] for r in res.results]


def _run_numpy(in_maps):
    """Host fallback mirroring the device math (for routing validation)."""
    outs = []
    for m in in_maps:
        x = m["xt"].astype(np.float32).T           # [tok, M]
        w1 = m["w1"].astype(np.float32)
        w2 = m["w2"].astype(np.float32)
        b1 = m["b1"].T.reshape(-1)                 # [H]
        h = x @ w1 + b1
        g = 0.5 * h * (1.0 + np.tanh(0.7978845608028654 * (h + 0.044715 * h ** 3)))
        outs.append((g @ w2).astype(np.float32))
    return outs


# --------------------------------------------------------------------------
# Entry point
# --------------------------------------------------------------------------

def kernel(hidden_states, gate_w, fc1_w, fc1_b, fc2_w, fc2_b):
    hidden_states = np.asarray(hidden_states)
    features = hidden_states.reshape(S, M)
    gate_w = np.asarray(gate_w)
    fc1_w, fc1_b = np.asarray(fc1_w), np.asarray(fc1_b)
    fc2_w, fc2_b = np.asarray(fc2_w), np.asarray(fc2_b)

    idx, keep, gate_val, l_aux, toks_per_e = _route(features, gate_w)

    # ---- shard: gather+transpose tokens per core, cast weights to bf16 ----
    in_maps = []
    for core in range(NCORES):
        e, half = core // 2, core % 2
        toks = toks_per_e[e]
        lo, hi = half * TOK, min((half + 1) * TOK, len(toks))
        xt = np.zeros((M, TOK), BF16)
        if hi > lo:
            xt[:, :hi - lo] = features[toks[lo:hi]].astype(BF16).T
        in_maps.append({
            "xt": xt,
            "w1": fc1_w[e].astype(BF16),
            "b1": np.ascontiguousarray(fc1_b[e].reshape(H // P, P).T.astype(np.float32)),
            "w2": fc2_w[e].astype(BF16),
        })

    if os.environ.get("MOE_NUMPY_MLP"):
        core_outs = _run_numpy(in_maps)
    else:
        core_outs = _run_device(in_maps)

    # ---- combine: scatter expert outputs back to token order ----
    out_full = np.zeros((S, O), np.float32)
    for e in range(E):
        toks = toks_per_e[e]
        n = len(toks)
        eo = np.concatenate([core_outs[2 * e], core_outs[2 * e + 1]], axis=0)[:n]
        eo = eo + fc2_b[e][None, :].astype(np.float32)
        out_full[toks] = gate_val[toks, None] * eo

    return out_full.reshape(B, T, O), l_aux
